# revision 13
# baseline (speedup 1.0000x reference)
"""2-layer GCN (Kipf) on 8 trn2 NeuronCores.

Sharding: nodes (rows of x / output) sharded across 8 cores; edges partitioned
by destination node; halo source features all-gathered per layer; weights
replicated.

Device pipeline per core (one NEFF, SPMD):
  A) H1_own = X_shard @ W1                       (TensorE, bf16)
  B) AllGather H1 -> H1_full (bf16, HBM)
  C) per 128-dst tile: dma_gather h1 rows of in-edges (sorted by dst tile /
     src range), build scatter matrix M[e,slot]=(iota==dstslot)*norm in one
     DVE op, matmul-accumulate M^T @ V into PSUM  => agg1; then
     relu(agg1+b1) -> transpose -> @W2 -> H2_own
  D) AllGather H2 -> H2_full
  E) same gather/scatter for layer 2 => agg2; +b2, log_softmax, write out.
"""
import os
import sys

sys.path.insert(0, "/opt/trn_rl_repo")

import numpy as np
import ml_dtypes

N_NODES, N_EDGES = 100000, 3200000
F_IN, F_HID, N_CLASS = 512, 256, 41
NCORES = 8
P = 128
NPC = N_NODES // NCORES            # 12500 nodes per core
NTILES = (NPC + P - 1) // P        # 98 dst tiles per core
LAST = NPC - (NTILES - 1) * P      # 84 rows in last tile
NRANGES = 4                        # src index ranges (int16-safe: < 32768)
RSZ = N_NODES // NRANGES           # 25000
CPAD = 64                          # classes padded for compute
HPAD = 128                         # h2 row padded to 128 bf16 = 256B rows
B2PAD = -30.0                      # pad-class logit bias

_cache = {}


def _prep(x, edge_index, W1, b1, W2, b2):
    src = np.asarray(edge_index[0], dtype=np.int64)
    dst = np.asarray(edge_index[1], dtype=np.int64)
    loop = np.arange(N_NODES, dtype=np.int64)
    src = np.concatenate([src, loop])
    dst = np.concatenate([dst, loop])
    deg = np.bincount(dst, minlength=N_NODES).astype(np.float32)
    dinv = deg ** -0.5
    norm = (dinv[src] * dinv[dst]).astype(np.float32)

    core = dst // NPC
    tile = (dst % NPC) // P
    slot = (dst % NPC) % P
    rng = src // RSZ

    order = np.lexsort((src, rng, tile, core))
    src, norm, core, tile, slot, rng = (
        a[order] for a in (src, norm, core, tile, slot, rng))

    R, T = NRANGES, NTILES
    key = (core * T + tile) * R + rng
    counts = np.bincount(key, minlength=NCORES * T * R).reshape(NCORES, T, R)
    Lmax = ((counts.max(axis=0) + P - 1) // P * P).astype(np.int64)  # [T, R]
    goff_flat = np.concatenate([[0], np.cumsum(Lmax.reshape(-1))])   # [T*R+1]
    E_pad = int(Lmax.sum())
    TC = E_pad // P                                # total chunks per core
    tile_coff = np.concatenate(
        [[0], np.cumsum(Lmax.sum(axis=1) // P)]).astype(np.int64)  # [T+1]

    cflat = counts.reshape(-1)
    gs = np.concatenate([[0], np.cumsum(cflat)])
    pos_in_group = np.arange(len(src)) - np.repeat(gs[:-1], cflat)
    flat_pos = goff_flat[tile * R + rng] + pos_in_group  # within-core position

    idx_local = (src - rng * RSZ).astype(np.int16)

    per_core = []
    for c in range(NCORES):
        m = core == c
        fp = flat_pos[m]
        idxp = np.zeros(E_pad, dtype=np.int16)
        slotp = np.zeros(E_pad, dtype=np.float32)
        normp = np.zeros(E_pad, dtype=np.float32)
        idxp[fp] = idx_local[m]
        slotp[fp] = slot[m]
        normp[fp] = norm[m]

        idx16 = np.tile(idxp.reshape(E_pad // 16, 16).T, (8, 1))  # [128, E/16]
        ds_t = np.ascontiguousarray(slotp.reshape(TC, P).T)       # [128, TC]
        nm_t = np.ascontiguousarray(normp.reshape(TC, P).T)        # [128, TC]
        per_core.append((idx16, ds_t, nm_t))

    x = np.asarray(x, dtype=np.float32)
    W1b = np.asarray(W1, dtype=np.float32).astype(ml_dtypes.bfloat16)
    W2p = np.zeros((F_HID, CPAD), dtype=np.float32)
    W2p[:, :N_CLASS] = np.asarray(W2, dtype=np.float32)
    W2p = W2p.astype(ml_dtypes.bfloat16)
    b1bc = np.broadcast_to(np.asarray(b1, dtype=np.float32), (P, F_HID)).copy()
    b2p = np.full(CPAD, B2PAD, dtype=np.float32)
    b2p[:N_CLASS] = np.asarray(b2, dtype=np.float32)
    b2bc = np.broadcast_to(b2p, (P, CPAD)).copy()
    iota = np.broadcast_to(np.arange(P, dtype=np.float32), (P, P)).astype(
        ml_dtypes.bfloat16).copy()

    in_maps = []
    for c in range(NCORES):
        idx16, ds_t, nm_t = per_core[c]
        xT = np.ascontiguousarray(
            x[c * NPC:(c + 1) * NPC, :].T).astype(ml_dtypes.bfloat16)
        in_maps.append({
            "xT": xT, "w1": W1b, "w2": W2p, "b1bc": b1bc, "b2bc": b2bc,
            "iota": iota, "idx16": idx16, "dstslot": ds_t, "normt": nm_t,
        })

    meta = (tuple(Lmax.reshape(-1).tolist()), E_pad, TC,
            tuple(tile_coff.tolist()))
    return in_maps, meta, Lmax, tile_coff, TC, E_pad


def _build(Lmax, tile_coff, TC, E_pad):
    import concourse.bacc as bacc
    import concourse.mybir as mybir
    import concourse.tile as tile
    from concourse import library_config
    from concourse.masks import make_identity

    bf16 = mybir.dt.bfloat16
    f32 = mybir.dt.float32
    i16 = mybir.dt.int16
    AF = mybir.ActivationFunctionType
    OP = mybir.AluOpType

    nc = bacc.Bacc("TRN2", target_bir_lowering=False, debug=False,
                   num_devices=NCORES)

    xT = nc.dram_tensor("xT", [F_IN, NPC], bf16, kind="ExternalInput")
    w1 = nc.dram_tensor("w1", [F_IN, F_HID], bf16, kind="ExternalInput")
    w2 = nc.dram_tensor("w2", [F_HID, CPAD], bf16, kind="ExternalInput")
    b1bc = nc.dram_tensor("b1bc", [P, F_HID], f32, kind="ExternalInput")
    b2bc = nc.dram_tensor("b2bc", [P, CPAD], f32, kind="ExternalInput")
    iota = nc.dram_tensor("iota", [P, P], bf16, kind="ExternalInput")
    idx16 = nc.dram_tensor("idx16", [P, E_pad // 16], i16, kind="ExternalInput")
    dstslot = nc.dram_tensor("dstslot", [P, TC], f32, kind="ExternalInput")
    normt = nc.dram_tensor("normt", [P, TC], f32, kind="ExternalInput")
    out = nc.dram_tensor("out", [NPC, N_CLASS], f32, kind="ExternalOutput")
    DBG = os.environ.get("GCN_DBG", "") == "1"
    if DBG:
        h1dbg = nc.dram_tensor("h1dbg", [NPC, F_HID], bf16,
                               kind="ExternalOutput")
        h2dbg = nc.dram_tensor("h2dbg", [NPC, HPAD], bf16,
                               kind="ExternalOutput")

    h1own = nc.dram_tensor("h1own", [NPC, F_HID], bf16)
    h1full = nc.dram_tensor("h1full", [N_NODES, F_HID], bf16,
                            addr_space="Shared")
    h2own = nc.dram_tensor("h2own", [NPC, HPAD], bf16)
    h2full = nc.dram_tensor("h2full", [N_NODES, HPAD], bf16,
                            addr_space="Shared")

    KT1 = F_IN // P   # 4 k-tiles for matmul1
    KT2 = F_HID // P  # 2 k-tiles for matmul2

    with tile.TileContext(nc) as tc:
        nc.gpsimd.load_library(library_config.mlp)
        with (
            tc.tile_pool(name="const", bufs=1) as cpool,
            tc.tile_pool(name="sbuf", bufs=2) as pool,
            tc.tile_pool(name="mpool", bufs=4) as mpool,
            tc.tile_pool(name="psum", bufs=2, space="PSUM") as psum,
        ):
            w1_sb = cpool.tile([P, KT1, F_HID], bf16)
            for kt in range(KT1):
                nc.sync.dma_start(out=w1_sb[:, kt, :],
                                  in_=w1[kt * P:(kt + 1) * P, :])
            w2_sb = cpool.tile([P, KT2, CPAD], bf16)
            for kt in range(KT2):
                nc.sync.dma_start(out=w2_sb[:, kt, :],
                                  in_=w2[kt * P:(kt + 1) * P, :])
            b1_sb = cpool.tile([P, F_HID], f32)
            nc.sync.dma_start(out=b1_sb[:], in_=b1bc[:])
            b2_sb = cpool.tile([P, CPAD], f32)
            nc.sync.dma_start(out=b2_sb[:], in_=b2bc[:])
            iota_sb = cpool.tile([P, P], bf16)
            nc.sync.dma_start(out=iota_sb[:], in_=iota[:])
            ident_sb = cpool.tile([P, P], bf16)
            make_identity(nc, ident_sb[:])

            PH = os.environ.get("GCN_PHASES", "ABCDE")
            # ---- Phase A: H1_own = X_shard @ W1 ----
            with nc.named_scope("matmul1"):
                for nb in range(NTILES if "A" in PH else 0):
                    rows = P if nb < NTILES - 1 else LAST
                    h1_ps = psum.tile([P, F_HID], f32, tag="agg")
                    for kt in range(KT1):
                        xt_sb = pool.tile([P, P], bf16, tag="xt")
                        nc.sync.dma_start(
                            out=xt_sb[:, :rows],
                            in_=xT[kt * P:(kt + 1) * P,
                                   nb * P:nb * P + rows])
                        nc.tensor.matmul(
                            h1_ps[:rows, :], xt_sb[:, :rows],
                            w1_sb[:, kt, :],
                            start=(kt == 0), stop=(kt == KT1 - 1))
                    h1_sb = pool.tile([P, F_HID], bf16, tag="h1sb")
                    nc.scalar.activation(h1_sb[:rows, :], h1_ps[:rows, :],
                                         AF.Copy)
                    nc.sync.dma_start(out=h1own[nb * P:nb * P + rows, :],
                                      in_=h1_sb[:rows, :])
                    if DBG:
                        nc.sync.dma_start(
                            out=h1dbg[nb * P:nb * P + rows, :],
                            in_=h1_sb[:rows, :])

            # ---- Phase B: AllGather H1 ----
            if "B" in PH:
                nc.gpsimd.collective_compute(
                    "AllGather", OP.bypass,
                    replica_groups=[list(range(NCORES))],
                    ins=[h1own[:]], outs=[h1full[:]])

            # ---- Phase C: layer-1 aggregate + relu + @W2 ----
            NOEQ = os.environ.get("GCN_NOEQ", "") == "1"
            NOEPI = os.environ.get("GCN_NOEPI", "") == "1"
            NOGATHER = os.environ.get("GCN_NOGATHER", "") == "1"

            def agg_tile(t, src_tbl, elem, v_tag, nch_t):
                """Gather in-edge rows for dst tile t and return the SBUF
                tile of gathered values [P, nch_t, elem]."""
                c0 = int(tile_coff[t])
                v_sb = pool.tile([P, nch_t, elem], bf16, tag=v_tag)
                if NOGATHER:
                    nc.vector.memset(v_sb[:], 0)
                    return v_sb
                idx_sb = pool.tile([P, nch_t * 8], i16, tag=v_tag + "ix")
                nc.sync.dma_start(
                    out=idx_sb[:],
                    in_=idx16[:, c0 * 8:(c0 + nch_t) * 8])
                cr = 0
                for r in range(NRANGES):
                    L = int(Lmax[t, r])
                    if L == 0:
                        continue
                    nch = L // P
                    nc.gpsimd.dma_gather(
                        v_sb[:, cr:cr + nch, :],
                        src_tbl[r * RSZ:(r + 1) * RSZ, :],
                        idx_sb[:, cr * 8:(cr + nch) * 8],
                        L, L, elem, elem_step=elem,
                        single_packet=(L <= 1024))
                    cr += nch
                return v_sb

            def m_chunks(t, nch_t, ds_sb, nm_sb):
                """Yield the per-chunk scatter matrices [P, P] bf16."""
                for c in range(nch_t):
                    m_bf = mpool.tile([P, P], bf16, tag="m")
                    if NOEQ:
                        nc.vector.memset(m_bf[:], 0)
                    else:
                        nc.vector.tensor_scalar(
                            m_bf[:], iota_sb[:], ds_sb[:, c:c + 1],
                            nm_sb[:, c:c + 1],
                            op0=OP.is_equal, op1=OP.mult)
                    yield c, m_bf

            NT_RUN = int(os.environ.get("GCN_TILES", NTILES))
            with nc.named_scope("layer1"):
                for t in range(NT_RUN if "C" in PH else 0):
                    rows = P if t < NTILES - 1 else LAST
                    c0 = int(tile_coff[t])
                    nch_t = int(tile_coff[t + 1]) - c0
                    ds_sb = pool.tile([P, nch_t], f32, tag="ds")
                    nc.sync.dma_start(out=ds_sb[:],
                                      in_=dstslot[:, c0:c0 + nch_t])
                    nm_sb = pool.tile([P, nch_t], f32, tag="nm")
                    nc.sync.dma_start(out=nm_sb[:],
                                      in_=normt[:, c0:c0 + nch_t])
                    v_sb = agg_tile(t, h1full, F_HID, "v1", nch_t)

                    agg_ps = psum.tile([P, F_HID], f32, tag="agg")
                    for c, m_bf in m_chunks(t, nch_t, ds_sb, nm_sb):
                        nc.tensor.matmul(
                            agg_ps[:], m_bf[:], v_sb[:, c, :],
                            start=(c == 0), stop=(c == nch_t - 1))

                    if NOEPI:
                        continue
                    # relu(agg + b1) in bf16
                    hsum = pool.tile([P, F_HID], f32, tag="hsum")
                    nc.vector.tensor_tensor(hsum[:], agg_ps[:], b1_sb[:],
                                            op=OP.add)
                    hrelu = pool.tile([P, F_HID], bf16, tag="hrelu")
                    nc.scalar.activation(hrelu[:], hsum[:], AF.Relu)
                    # transpose [P, 256] -> 2 x [P, P] (k-major)
                    hT = pool.tile([P, KT2, P], bf16, tag="ht")
                    for kt in range(KT2):
                        t_ps = psum.tile([P, P], bf16, tag="tps")
                        nc.tensor.transpose(
                            out=t_ps[:], in_=hrelu[:, kt * P:(kt + 1) * P],
                            identity=ident_sb[:])
                        nc.vector.tensor_copy(hT[:, kt, :], t_ps[:])
                    h2_ps = psum.tile([P, CPAD], f32, tag="h2ps")
                    for kt in range(KT2):
                        nc.tensor.matmul(
                            h2_ps[:rows, :], hT[:, kt, :rows],
                            w2_sb[:, kt, :],
                            start=(kt == 0), stop=(kt == KT2 - 1))
                    h2t = pool.tile([P, HPAD], bf16, tag="h2t")
                    nc.vector.memset(h2t[:, CPAD:], 0)
                    nc.scalar.activation(h2t[:rows, :CPAD], h2_ps[:rows, :],
                                         AF.Copy)
                    nc.sync.dma_start(out=h2own[t * P:t * P + rows, :],
                                      in_=h2t[:rows, :])
                    if DBG:
                        nc.sync.dma_start(
                            out=h2dbg[t * P:t * P + rows, :],
                            in_=h2t[:rows, :])

            # ---- Phase D: AllGather H2 ----
            if "D" in PH:
                nc.gpsimd.collective_compute(
                    "AllGather", OP.bypass,
                    replica_groups=[list(range(NCORES))],
                    ins=[h2own[:]], outs=[h2full[:]])

            # ---- Phase E: layer-2 aggregate + b2 + log_softmax ----
            with nc.named_scope("layer2"):
                for t in range(NT_RUN if "E" in PH else 0):
                    rows = P if t < NTILES - 1 else LAST
                    c0 = int(tile_coff[t])
                    nch_t = int(tile_coff[t + 1]) - c0
                    ds_sb = pool.tile([P, nch_t], f32, tag="ds")
                    nc.sync.dma_start(out=ds_sb[:],
                                      in_=dstslot[:, c0:c0 + nch_t])
                    nm_sb = pool.tile([P, nch_t], f32, tag="nm")
                    nc.sync.dma_start(out=nm_sb[:],
                                      in_=normt[:, c0:c0 + nch_t])
                    v_sb = agg_tile(t, h2full, HPAD, "v2", nch_t)

                    out_ps = psum.tile([P, CPAD], f32, tag="h2ps")
                    for c, m_bf in m_chunks(t, nch_t, ds_sb, nm_sb):
                        nc.tensor.matmul(
                            out_ps[:], m_bf[:], v_sb[:, c, 0:CPAD],
                            start=(c == 0), stop=(c == nch_t - 1))

                    logit = pool.tile([P, CPAD], f32, tag="lg")
                    nc.vector.tensor_tensor(logit[:], out_ps[:], b2_sb[:],
                                            op=OP.add)
                    mx = pool.tile([P, 1], f32, tag="mx")
                    nc.vector.tensor_reduce(mx[:], logit[:],
                                            axis=mybir.AxisListType.X,
                                            op=OP.max)
                    xm = pool.tile([P, CPAD], f32, tag="xm")
                    nc.vector.tensor_scalar(xm[:], logit[:], mx[:, 0:1],
                                            None, op0=OP.subtract)
                    ex = pool.tile([P, CPAD], f32, tag="ex")
                    se = pool.tile([P, 1], f32, tag="se")
                    nc.scalar.activation(ex[:], xm[:], AF.Exp,
                                         accum_out=se[:, 0:1])
                    ls = pool.tile([P, 1], f32, tag="ls")
                    nc.scalar.activation(ls[:], se[:], AF.Ln)
                    res = pool.tile([P, CPAD], f32, tag="res")
                    nc.vector.tensor_scalar(res[:], xm[:], ls[:, 0:1],
                                            None, op0=OP.subtract)
                    nc.sync.dma_start(
                        out=out[t * P:t * P + rows, :],
                        in_=res[:rows, 0:N_CLASS])

    nc.compile()
    return nc


def _run(inputs, trace=False, tmpdir=None):
    from concourse.bass_utils import run_bass_kernel_spmd

    in_maps, meta, Lmax, tile_coff, TC, E_pad = _prep(**inputs)
    if meta not in _cache:
        _cache[meta] = _build(Lmax, tile_coff, TC, E_pad)
    nc = _cache[meta]
    res = run_bass_kernel_spmd(nc, in_maps, core_ids=list(range(NCORES)),
                               trace=trace, tmpdir=tmpdir)
    outp = np.concatenate([res.results[c]["out"] for c in range(NCORES)],
                          axis=0)
    return outp.astype(np.float32), res


def kernel(x, edge_index, W1, b1, W2, b2):
    outp, _ = _run(dict(x=x, edge_index=edge_index, W1=W1, b1=b1,
                        W2=W2, b2=b2))
    return outp
